# revision 2
# baseline (speedup 1.0000x reference)
"""Trainium2 Bass kernel for multi-head attention (nn_Attention_61168924230279).

Module: y = Attention(x) with q/k/v/o linear layers (eqx convention
y = x @ W.T + b), NeoX-style RoPE on q and k, softmax attention.
  x [2, 2048, 2048], 16 heads x 128 head_dim, fp32.

Sharding: tensor-parallel over heads. 8 cores x 2 heads each.
Core c owns feature slice [256c, 256c+256) of wq/wk/wv rows and wo cols.
Each core computes a partial y; partials are summed on the host (+bo,
+bv@wo.T which is dropped on device since attn rows sum to 1).

v2 design (vs fp32r baseline):
  - all matmuls in bf16 (same 1 cycle/row as fp32r but FWL weight loads,
    half DMA/SBUF, 2x DVE elementwise)
  - exp outputs (at) and V in fp8-e4m3; the l row-sum and attn@V matmuls
    use DoubleRow perf mode (0.5 cycles/row, K=256 per instruction)
  - V optionally split V = fp8(V) + fp8(V - fp8(V)) for accuracy
  - bias added on PE via K=1 ones matmul; rope reads an ACT-evicted bf16
    copy of the projection psum
  - x prepacked on host so each xt DMA is 4KB/partition contiguous
  - y written per (b, m-tile) as one [128, 2048] fp16 DMA on the Pool
    (SWDGE) queue; psum evicts alternate DVE/ACT
  - phase B (attention) chunks are pulled greedily into the projection
    stream as their QT/KT/V dependencies become ready, across batches
"""

import math
import os
from contextlib import ExitStack

import numpy as np

import concourse.bass as bass
import concourse.tile as tile
from concourse import bacc, mybir
from concourse.masks import make_identity

P = 128
D = 2048
ND = D // P            # 16 contraction chunks
B = 2
S = 2048
M = B * S              # 4096
NMT = S // P           # 16 m-tiles per batch
HD = 128
HPC = 2                # heads per core
E2 = HPC * HD          # 256 (v width per core)
E4 = 2 * E2            # 512 (q|k width per core)
NQC = S // 512         # 4 query chunks per batch
NKT = S // P           # 16 key tiles per batch
NKP = NKT // 2         # 8 key-tile pairs (fp8 DoubleRow granularity)
N_CORES = 8
SCALE = 1.0 / math.sqrt(HD)
ROPE_THETA = 10000.0

F32 = mybir.dt.float32
BF16 = mybir.dt.bfloat16
F16 = mybir.dt.float16
F8 = mybir.dt.float8e4

AT8 = os.environ.get("K_AT8", "1") == "1"        # fp8 at + DoubleRow l/av
VSPLIT = os.environ.get("K_VSPLIT", "1") == "1"  # V = v8 + fp8(v - v8)
WO8 = os.environ.get("K_WO8", "0") == "1"        # fp8 outc/wo DoubleRow
CH = int(os.environ.get("K_CH", "1"))            # transpose lag (m-tiles)
PB = int(os.environ.get("K_PB", "3"))            # attn pairs pulled per m-tile
DR = mybir.MatmulPerfMode.DoubleRow


def _emit(nc, tc, t):
    with ExitStack() as ctx:
        ec = ctx.enter_context
        const = ec(tc.tile_pool(name="const", bufs=1))
        wpool = ec(tc.tile_pool(name="weights", bufs=1))
        tabs = ec(tc.tile_pool(name="tables", bufs=1))
        xtp = ec(tc.tile_pool(name="xt", bufs=int(os.environ.get("K_XTB", "3"))))
        qk0p = ec(tc.tile_pool(name="qk0", bufs=2))
        qkp = ec(tc.tile_pool(name="qk", bufs=int(os.environ.get("K_QKB", "3"))))
        up = ec(tc.tile_pool(name="u", bufs=2))
        qtkv = ec(tc.tile_pool(name="qtkv", bufs=1))
        atp = ec(tc.tile_pool(name="at", bufs=int(os.environ.get("K_ATB", "4"))))
        rbp = ec(tc.tile_pool(name="rb", bufs=2))
        rsp = ec(tc.tile_pool(name="rs", bufs=2))
        outp = ec(tc.tile_pool(name="outT", bufs=2))
        yp = ec(tc.tile_pool(name="y", bufs=int(os.environ.get("K_YB", "2"))))
        psA = ec(tc.tile_pool(name="psA", bufs=int(os.environ.get("K_PSA", "5")), space="PSUM"))
        psAcc = ec(tc.tile_pool(name="psAcc", bufs=2, space="PSUM"))
        psL = ec(tc.tile_pool(name="psL", bufs=1, space="PSUM"))

        # --- constants / weights / tables (loaded once) ---
        ones_row = const.tile([1, P], BF16)
        nc.vector.memset(ones_row, 1.0)
        ident = const.tile([P, P], F32)
        make_identity(nc, ident)
        identr = const.tile([P, P], BF16)
        nc.vector.tensor_copy(identr, ident)
        at_dt = F8 if AT8 else BF16
        ones2 = const.tile([P, 2, 16], at_dt)
        nc.vector.memset(ones2, 1.0)
        bqk_t = const.tile([1, E4], BF16)
        nc.sync.dma_start(bqk_t, t["bqk"])

        wqk_s = wpool.tile([P, ND, E4], BF16)
        nc.gpsimd.dma_start(wqk_s, t["wqkT"].rearrange("(k p) e -> p k e", p=P))
        wv_s = wpool.tile([P, ND, E2], BF16)
        nc.gpsimd.dma_start(wv_s, t["wvT"].rearrange("(k p) e -> p k e", p=P))
        wo_dt = F8 if WO8 else BF16
        wo_s = wpool.tile([P, HPC, D], wo_dt)
        if WO8:
            wo_sb = wpool.tile([P, HPC, D], BF16)
            nc.gpsimd.dma_start(wo_sb, t["woT"].rearrange("(h p) d -> p h d", p=P))
            nc.vector.tensor_copy(wo_s, wo_sb)
        else:
            nc.gpsimd.dma_start(wo_s, t["woT"].rearrange("(h p) d -> p h d", p=P))
        cos_s = tabs.tile([P, NMT, HD], BF16)
        nc.sync.dma_start(cos_s, t["cos"].rearrange("(m p) e -> p m e", p=P))
        sin_s = tabs.tile([P, NMT, HD], BF16)
        nc.sync.dma_start(sin_s, t["sins"].rearrange("(m p) e -> p m e", p=P))

        def bc4(apin, count, width, off=0):
            # [128, width] AP -> [128, (count bcast), width] stride-0 view
            a = apin
            return bass.AP(tensor=a.tensor, offset=a.offset + off,
                           ap=[list(a.ap[0]), [0, count], [1, width]])

        reps = int(os.environ.get("K_REPS", "1"))
        for rep in range(reps):
            # per-batch persistent tiles
            QT = {}
            KT = {}
            V8 = {}
            VLO = {}
            for b in range(B):
                QT[b] = qtkv.tile([P, HPC, S], BF16, tag=f"QT{b}")
                KT[b] = qtkv.tile([P, HPC, S], BF16, tag=f"KT{b}")
                V8[b] = qtkv.tile([P, NMT, E2], at_dt, tag=f"V{b}")
                if AT8 and VSPLIT:
                    VLO[b] = qtkv.tile([P, NMT, E2], F8, tag=f"VL{b}")

            qk_tiles = {}

            def emit_proj(b, mt):
                xt = xtp.tile([P, ND, P], BF16, tag="xt")
                nc.sync.dma_start(xt, t["xP"][:, b * NMT + mt, :, :])
                qk_ps = psA.tile([P, E4], F32, tag="ps", name=f"qk_{b}_{mt}")
                for k in range(ND):
                    nc.tensor.matmul(qk_ps, xt[:, k, :], wqk_s[:, k, :],
                                     start=(k == 0), stop=False)
                nc.tensor.matmul(qk_ps, ones_row, bqk_t, start=False, stop=True)
                v_ps = psA.tile([P, E2], F32, tag="ps", name=f"v_{b}_{mt}")
                for k in range(ND):
                    nc.tensor.matmul(v_ps, xt[:, k, :], wv_s[:, k, :],
                                     start=(k == 0), stop=(k == ND - 1))
                nc.scalar.copy(V8[b][:, mt, :], v_ps)
                if AT8 and VSPLIT:
                    nc.vector.tensor_sub(VLO[b][:, mt, :], v_ps, V8[b][:, mt, :])
                qk0 = qk0p.tile([P, E4], BF16, tag="qk0")
                nc.scalar.copy(qk0, qk_ps)

                qk = qkp.tile([P, E4], BF16, tag="qk")
                qk_tiles[(b, mt)] = qk
                u = up.tile([P, E4], BF16, tag="u")
                qk4 = qk.rearrange("p (g e) -> p g e", g=4)
                q04 = qk0.rearrange("p (g e) -> p g e", g=4)
                u4 = u.rearrange("p (g e) -> p g e", g=4)
                cosm = cos_s[:, mt, :]
                sinm = sin_s[:, mt, :]
                H2 = HD // 2
                nc.vector.tensor_mul(qk4, q04, bc4(cosm, 4, HD))
                nc.vector.tensor_mul(u4[:, :, 0:H2], q04[:, :, H2:HD], bc4(sinm, 4, H2))
                nc.vector.tensor_mul(u4[:, :, H2:HD], q04[:, :, 0:H2], bc4(sinm, 4, H2, off=H2))
                nc.vector.tensor_add(qk, qk, u)

            def emit_transpose(b, mt):
                qk4 = qk_tiles.pop((b, mt)).rearrange("p (g e) -> p g e", g=4)
                tp_ps = psA.tile([P, E4], BF16, tag="ps", name=f"tp_{b}_{mt}")
                for g in range(4):
                    nc.tensor.transpose(tp_ps[:, g * P:(g + 1) * P], qk4[:, g, :], identr)
                msl = slice(mt * P, (mt + 1) * P)
                nc.vector.tensor_copy(
                    QT[b][:, :, msl], tp_ps[:, 0:E2].rearrange("p (h e) -> p h e", h=HPC))
                nc.vector.tensor_copy(
                    KT[b][:, :, msl], tp_ps[:, E2:E4].rearrange("p (h e) -> p h e", h=HPC))

            # ---- phase B state ----
            streams = [(b, qc) for b in range(B) for qc in range(NQC)]
            sstate = {s: {"pairs": 0, "final": False, "l": None, "av": {}} for s in streams}
            proj_done = {b: 0 for b in range(B)}
            trans_done = {b: 0 for b in range(B)}

            def emit_pair(b, qc, p):
                st = sstate[(b, qc)]
                qsl = slice(qc * 512, (qc + 1) * 512)
                at2 = atp.tile([P, 2, HPC, E4], at_dt, tag="at", name=f"at_{b}_{qc}_{p}")
                for kk in range(2):
                    kt = 2 * p + kk
                    for h in range(HPC):
                        lg = psA.tile([P, E4], F32, tag="ps", name=f"lg_{b}_{qc}_{kt}_{h}")
                        nc.tensor.matmul(lg, KT[b][:, h, kt * P:(kt + 1) * P],
                                         QT[b][:, h, qsl], start=True, stop=True)
                        nc.scalar.activation(at2[:, kk, h, :], lg,
                                             mybir.ActivationFunctionType.Exp, scale=SCALE)
                if p == 0:
                    st["l"] = psL.tile([33, E4], F32, tag="l", name=f"l_{b}_{qc}")
                    for h in range(HPC):
                        st["av"][h] = psAcc.tile([P, E4], F32, tag="av", name=f"av_{b}_{qc}_{h}")
                if AT8:
                    for h in range(HPC):
                        nc.tensor.matmul(st["l"][32 * h:32 * h + 1, :], ones2[:, :, 0:1],
                                         at2[:, :, h, :],
                                         start=(p == 0), stop=(p == NKP - 1), perf_mode=DR)
                    for h in range(HPC):
                        nc.tensor.matmul(st["av"][h], V8[b][:, 2 * p:2 * p + 2, h * HD:(h + 1) * HD],
                                         at2[:, :, h, :],
                                         start=(p == 0), stop=(p == NKP - 1 and not VSPLIT),
                                         perf_mode=DR)
                        if VSPLIT:
                            nc.tensor.matmul(st["av"][h], VLO[b][:, 2 * p:2 * p + 2, h * HD:(h + 1) * HD],
                                             at2[:, :, h, :],
                                             start=False, stop=(p == NKP - 1), perf_mode=DR)
                else:
                    for kk in range(2):
                        for h in range(HPC):
                            nc.tensor.matmul(st["l"][32 * h:32 * h + 1, :], ones2[:, 0, 0:1],
                                             at2[:, kk, h, :],
                                             start=(p == 0 and kk == 0), stop=(p == NKP - 1 and kk == 1))
                    for kk in range(2):
                        kt = 2 * p + kk
                        for h in range(HPC):
                            nc.tensor.matmul(st["av"][h], V8[b][:, kt, h * HD:(h + 1) * HD],
                                             at2[:, kk, h, :],
                                             start=(p == 0 and kk == 0), stop=(p == NKP - 1 and kk == 1))
                st["pairs"] += 1

            def emit_final(b, qc):
                st = sstate[(b, qc)]
                outc = outp.tile([P, HPC, E4], wo_dt, tag="outc", name=f"outc_{b}_{qc}")
                for h in range(HPC):
                    rs = rsp.tile([1, E4], F32, tag="rs", name=f"rs_{b}_{qc}_{h}")
                    nc.vector.reciprocal(rs, st["l"][32 * h:32 * h + 1, :])
                    rb = rbp.tile([P, E4], F32, tag="rb", name=f"rb_{b}_{qc}_{h}")
                    nc.gpsimd.partition_broadcast(rb, rs)
                    nc.vector.tensor_mul(outc[:, h, :], st["av"][h], rb)
                for mtl in range(4):
                    mt = qc * 4 + mtl
                    lsl = slice(mtl * P, (mtl + 1) * P)
                    yt = yp.tile([P, D], F16, tag="yt", name=f"yt_{b}_{mt}")
                    for oc in range(4):
                        osl = slice(oc * 512, (oc + 1) * 512)
                        y_ps = psA.tile([P, 512], F32, tag="ps", name=f"yp_{b}_{mt}_{oc}")
                        if WO8:
                            nc.tensor.matmul(y_ps, outc[:, :, lsl], wo_s[:, :, osl],
                                             start=True, stop=True, perf_mode=DR)
                        else:
                            nc.tensor.matmul(y_ps, outc[:, 0, lsl], wo_s[:, 0, osl],
                                             start=True, stop=False)
                            nc.tensor.matmul(y_ps, outc[:, 1, lsl], wo_s[:, 1, osl],
                                             start=False, stop=True)
                        if oc % 2 == 0:
                            nc.vector.tensor_copy(yt[:, osl], y_ps)
                        else:
                            nc.scalar.copy(yt[:, osl], y_ps)
                    nc.gpsimd.dma_start(
                        t["y"][b * S + mt * P: b * S + (mt + 1) * P, :], yt)
                st["final"] = True

            def drain(budget):
                while budget > 0:
                    s = None
                    for cand in streams:
                        if not sstate[cand]["final"]:
                            s = cand
                            break
                    if s is None:
                        return
                    b, qc = s
                    st = sstate[s]
                    if st["pairs"] < NKP:
                        if trans_done[b] < 4 * qc + 4:
                            return
                        if proj_done[b] < 2 * (st["pairs"] + 1):
                            return
                        emit_pair(b, qc, st["pairs"])
                        budget -= 1
                    else:
                        emit_final(b, qc)
                        budget -= 2

            # ---- main emission ----
            for b in range(B):
                for mt in range(NMT):
                    emit_proj(b, mt)
                    proj_done[b] += 1
                    if mt >= CH:
                        emit_transpose(b, mt - CH)
                        trans_done[b] += 1
                    drain(PB)
                for mt in range(NMT - CH, NMT):
                    emit_transpose(b, mt)
                    trans_done[b] += 1
                drain(PB)
            drain(1 << 30)


def build_program():
    nc = bacc.Bacc(
        "TRN2",
        target_bir_lowering=False,
        debug=False,
        enable_asserts=False,
        num_devices=N_CORES,
    )
    t = {
        "xP": nc.dram_tensor("xP", [P, B * NMT, ND, P], BF16, kind="ExternalInput").ap(),
        "wqkT": nc.dram_tensor("wqkT", [D, E4], BF16, kind="ExternalInput").ap(),
        "wvT": nc.dram_tensor("wvT", [D, E2], BF16, kind="ExternalInput").ap(),
        "woT": nc.dram_tensor("woT", [E2, D], BF16, kind="ExternalInput").ap(),
        "bqk": nc.dram_tensor("bqk", [1, E4], BF16, kind="ExternalInput").ap(),
        "cos": nc.dram_tensor("cos", [S, HD], BF16, kind="ExternalInput").ap(),
        "sins": nc.dram_tensor("sins", [S, HD], BF16, kind="ExternalInput").ap(),
        "y": nc.dram_tensor("y", [M, D], F16, kind="ExternalOutput").ap(),
    }
    with tile.TileContext(nc) as tc:
        _emit(nc, tc, t)
    nc.compile()
    return nc


def rope_tables():
    inv_freq = 1.0 / (ROPE_THETA ** (np.arange(0, HD, 2, dtype=np.float32) / HD))
    angles = np.outer(np.arange(S, dtype=np.float32), inv_freq)
    ang = np.concatenate([angles, angles], axis=-1)
    cos = np.cos(ang).astype(np.float32)
    sin = np.sin(ang).astype(np.float32)
    sins = np.concatenate([-sin[:, :64], sin[:, 64:]], axis=-1)
    return cos, sins


def make_in_maps(x, wq, bq, wk, bk, wv, bv, wo, bo):
    import ml_dtypes
    bf = ml_dtypes.bfloat16
    xf = np.asarray(x, dtype=np.float32).reshape(M, D)
    # xP[p, g, k, j] = x[g*128+j, k*128+p]
    xP = np.ascontiguousarray(
        xf.reshape(B * NMT, P, ND, P).transpose(3, 0, 2, 1).astype(bf))
    cos, sins = rope_tables()
    maps = []
    for c in range(N_CORES):
        sl = slice(c * E2, (c + 1) * E2)
        maps.append({
            "xP": xP,
            "wqkT": np.ascontiguousarray(
                np.concatenate([wq[sl], wk[sl]], axis=0).T.astype(bf)),
            "wvT": np.ascontiguousarray(wv[sl].T.astype(bf)),
            "woT": np.ascontiguousarray(wo[:, sl].T.astype(bf)),
            "bqk": np.concatenate([bq[sl], bk[sl]])[None, :].astype(bf),
            "cos": cos.astype(bf),
            "sins": sins.astype(bf),
        })
    return maps


_NC = None


def kernel(**inputs) -> np.ndarray:
    global _NC
    inputs = {k: np.ascontiguousarray(np.asarray(v, dtype=np.float32))
              for k, v in inputs.items()}
    if _NC is None:
        _NC = build_program()
    from concourse.bass_utils import run_bass_kernel_spmd

    maps = make_in_maps(**inputs)
    res = run_bass_kernel_spmd(_NC, maps, list(range(N_CORES)))
    y = np.zeros((M, D), np.float64)
    for c in range(N_CORES):
        y += res.results[c]["y"].astype(np.float64)
    y += inputs["bo"][None, :] + (inputs["bv"].astype(np.float64) @ inputs["wo"].T.astype(np.float64))[None, :]
    return y.astype(np.float32).reshape(B, S, D)
